# revision 12
# baseline (speedup 1.0000x reference)
"""Trainium2 Bass kernel for nn_Extract_HyperSpherePrototypes.

Computation (see reference):
  1. L2-normalize each pixel's feature vector over the channel dim F=256.
  2. Segment-sum normalized features by label into [C+1=20, F] prototypes.
  3. Drop void class, transpose to [F, 19], L2-normalize each column.

Sharding: data-parallel over batch (16 items / 8 cores = 2 per core).
Each core computes a local [20, 256] partial, AllReduce(sum) across the
8 cores, then every core normalizes columns and writes the full output.

Per-core layout trick: features[b] is loaded as tiles [h=128; f, w]
(partition dim = h, free dims = (f-chunk, w)) which keeps DMA reads
contiguous in 512B runs. The per-pixel inverse norm is folded into the
one-hot matrix M[h, w, c] = (label==c) * rsqrt(sum_f x^2), so the
segment-sum becomes, per w column, a matmul contracting h:
  protos[c, f] += sum_h M[h, w, c] * X[h, f, w]
"""

import numpy as np

import concourse.bass as bass
import concourse.bacc as bacc
import concourse.tile as tile
from concourse import mybir
from concourse.bass_utils import run_bass_kernel_spmd
from concourse.tile import TileContext

F32 = mybir.dt.float32
AX = mybir.AxisListType
OP = mybir.AluOpType
ACT_FN = mybir.ActivationFunctionType

NCORES = 8
B_TOT = 16
BPC = B_TOT // NCORES  # batches per core
F = 256
H = 128
W = 128
C = 20  # 19 known + void
FC = 64  # f-chunk size
NFC = F // FC
WH = 64  # w-half for square scratch

EPS2 = 1e-24  # matches max(norm, 1e-12) in the reference

import os
_NO_CC = bool(int(os.environ.get("KERNEL_NO_CC", "0")))


def _bcast_ap(t, dims):
    """Manual broadcast AP: dims is a list of [stride, count] pairs."""
    ap = t[:] if not isinstance(t, bass.AP) else t
    return bass.AP(tensor=ap.tensor, offset=ap.offset, ap=dims)


def build_nc():
    nc = bacc.Bacc("TRN2", target_bir_lowering=False)

    feats = nc.declare_dram_parameter("feats", [BPC, F, H, W], F32, isOutput=False)
    labs = nc.declare_dram_parameter("labs", [BPC, H, W], F32, isOutput=False)
    out_d = nc.declare_dram_parameter("out", [F, C - 1], F32, isOutput=True)

    cc_in = nc.dram_tensor("cc_in", [C, F], F32)
    cc_out = nc.dram_tensor("cc_out", [C, F], F32, addr_space="Shared")


    with TileContext(nc) as tc:
        with (
            tc.tile_pool(name="consts", bufs=1) as consts,
            tc.tile_pool(name="xp", bufs=4) as xp,
            tc.tile_pool(name="sqp", bufs=1) as sqp,
            tc.tile_pool(name="mp", bufs=2) as mp,
            tc.tile_pool(name="normp", bufs=2) as normp,
            tc.tile_pool(name="finp", bufs=1) as finp,
            tc.tile_pool(name="psum", bufs=1, space="PSUM") as psum,
        ):
            iota_i = consts.tile([H, C], mybir.dt.int32)
            nc.gpsimd.iota(iota_i, pattern=[[1, C]], base=0, channel_multiplier=0)
            iota_sb = consts.tile([H, C], F32)
            nc.vector.tensor_copy(iota_sb, iota_i)
            eps_sb = consts.tile([H, 1], F32)
            nc.vector.memset(eps_sb, EPS2)

            protos_ps = psum.tile([C, F], F32)

            feats_ap = feats.ap()
            labs_ap = labs.ap()

            for b in range(BPC):
                lab_sb = normp.tile([H, W], F32)
                nc.sync.dma_start(out=lab_sb, in_=labs_ap[b])

                ssq4 = normp.tile([H, W, NFC], F32)
                xts = []
                for fc in range(NFC):
                    xt = xp.tile([H, FC, W], F32)
                    nc.sync.dma_start(
                        out=xt,
                        in_=feats_ap[b, fc * FC : (fc + 1) * FC].rearrange(
                            "f h w -> h f w"
                        ),
                    )
                    xts.append(xt)
                    for wh in range(W // WH):
                        sq = sqp.tile([H, FC, WH], F32)
                        nc.scalar.activation(
                            out=sq,
                            in_=xt[:, :, wh * WH : (wh + 1) * WH],
                            func=ACT_FN.Square,
                        )
                        nc.vector.tensor_reduce(
                            out=ssq4[:, wh * WH : (wh + 1) * WH, fc],
                            in_=sq.rearrange("h f w -> h w f"),
                            axis=AX.X,
                            op=OP.add,
                        )

                # sum over the NFC chunk partials -> [H, W]
                ssq = normp.tile([H, W], F32)
                nc.vector.tensor_reduce(out=ssq, in_=ssq4, axis=AX.X, op=OP.add)
                # inv = 1/sqrt(ssq + eps)
                nc.scalar.activation(out=ssq, in_=ssq, func=ACT_FN.Sqrt, bias=eps_sb[:])
                inv = normp.tile([H, W], F32)
                nc.vector.reciprocal(out=inv, in_=ssq)

                # M[h, w, c] = (iota_c == lab) * inv
                m_sb = mp.tile([H, W, C], F32)
                nc.vector.tensor_tensor(
                    out=m_sb,
                    in0=_bcast_ap(iota_sb, [iota_sb[:].ap[0], [0, W], [1, C]]),
                    in1=lab_sb[:].to_broadcast([H, W, C]),
                    op=OP.is_equal,
                )
                nc.vector.tensor_tensor(
                    out=m_sb,
                    in0=m_sb,
                    in1=inv[:].to_broadcast([H, W, C]),
                    op=OP.mult,
                )

                # segment-sum: per w, protos[c, fc*FC:+FC] += M[:,w,:].T @ X[:,:,w]
                for fc in range(NFC):
                    for w in range(W):
                        nc.tensor.matmul(
                            out=protos_ps[:, fc * FC : (fc + 1) * FC],
                            lhsT=m_sb[:, w, :],
                            rhs=xts[fc][:, :, w],
                            start=(b == 0 and fc == 0 and w == 0),
                            stop=(b == BPC - 1 and fc == NFC - 1 and w == W - 1),
                        )

            # local partial -> DRAM -> AllReduce -> back
            protos_sb = finp.tile([C, F], F32)
            nc.scalar.copy(out=protos_sb, in_=protos_ps)
            if not _NO_CC:
                nc.sync.dma_start(out=cc_in.ap(), in_=protos_sb)
                nc.gpsimd.collective_compute(
                    "AllReduce",
                    OP.add,
                    ins=[cc_in.ap().opt()],
                    outs=[cc_out.ap().opt()],
                    replica_groups=[list(range(NCORES))],
                )
                red_sb = finp.tile([C, F], F32)
                nc.sync.dma_start(out=red_sb, in_=cc_out.ap())
            else:
                red_sb = protos_sb

            # column norms (per class over F): pn2[c] = sum_f red[c,f]^2
            scr = finp.tile([C, F], F32)
            pn = finp.tile([C, 1], F32)
            nc.vector.tensor_mul(scr, red_sb, red_sb)
            nc.vector.tensor_reduce(out=pn, in_=scr, axis=AX.X, op=OP.add)
            nc.scalar.activation(out=pn, in_=pn, func=ACT_FN.Sqrt, bias=eps_sb[:C])
            pninv = finp.tile([C, 1], F32)
            nc.vector.reciprocal(out=pninv, in_=pn)
            nc.vector.tensor_scalar_mul(out=red_sb, in0=red_sb, scalar1=pninv)

            # transposed write: out[f, c] = red_sb[c, f]
            o_ap = out_d.ap()
            nc.sync.dma_start(
                out=bass.AP(tensor=o_ap.tensor, offset=o_ap.offset,
                            ap=[[1, C - 1], [C - 1, F]]),
                in_=red_sb[0 : C - 1, :],
            )

    nc.compile()
    return nc


_NC_CACHE = None


def _get_nc():
    global _NC_CACHE
    if _NC_CACHE is None:
        _NC_CACHE = build_nc()
    return _NC_CACHE


def kernel(features: np.ndarray, labels: np.ndarray) -> np.ndarray:
    features = np.ascontiguousarray(np.asarray(features, dtype=np.float32))
    labs_f32 = np.asarray(labels, dtype=np.float32)  # values 0..19, exact in f32

    nc = _get_nc()
    in_maps = []
    for core in range(NCORES):
        in_maps.append(
            {
                "feats": features[core * BPC : (core + 1) * BPC],
                "labs": np.ascontiguousarray(labs_f32[core * BPC : (core + 1) * BPC]),
            }
        )
    res = run_bass_kernel_spmd(nc, in_maps, core_ids=list(range(NCORES)))
    return np.asarray(res.results[0]["out"], dtype=np.float32)
